# revision 27
# baseline (speedup 1.0000x reference)
"""DiT block (Linformer attention + adaLN + MLP) on 8 TRN2 NeuronCores.

Sharding: data-parallel over batch (B=8 -> one batch element per core).

v2 design notes:
 - adaLN conditioning folded into weights: wq/wk/wv rows scaled by scale1
   during their f32->f16 resident cast (tensor_scalar = same cost as the
   plain cast), m1w rows scaled by scale2 during its streamed cast.
   LayerNorm emits pure xn in f16; offsets enter via per-partition biases:
   qT gets (wq^T o1 + bq) columns (one PE GEMV), P_EF gets a rank-1
   (o1/s1) x colsum(E|F) correction fused into its PSUM evacuation
   (scalar_tensor_tensor), x2T gets (o2/s2) columns at transpose-evac.
 - Ew|Fw streamed once; colsums accumulate in the same pass; the f16 copy
   is kept in SBUF for the second PEF j-half pass.
 - Attention: the A@V stationary operand carries V in cols 0:64 and ONES
   in cols 64:128 of each head block, so softmax denominators land on PSUM
   partitions 64:128 at no extra PE cost.  Evac denominators (scalar),
   reciprocal on SBUF (DVE), multiply PSUM x SBUF -> normalized aoT f16
   (DVE).  No PE broadcasts, no single-partition row ops.
 - wo + residual + LN2 bn_stats fused per token group right after that
   group's attention pairs; attention residual kept in SBUF (no DRAM
   round-trip).
 - MLP in 2 supergroups of 1024 tokens: m1w/m2w streamed fp32 and cast
   inline to f16 (m1 cast fused with the scale2 fold), gelu emits f16 hm,
   m2 accumulates 8 PSUM banks per D-half, residual added from SBUF.
 - Emission interleaves LN1 groups with the conditioning GEMV layers so
   the DMA queues serve x and h1w/g1w before the bulk weight streams.
"""
import contextlib

import numpy as np

import concourse.bass as bass
import concourse.mybir as mybir
import concourse.tile as tile
from concourse import bacc
from concourse.bass import ds
from concourse.bass_utils import run_bass_kernel_spmd
from concourse.masks import make_identity

f32 = mybir.dt.float32
f16 = mybir.dt.float16
AF = mybir.ActivationFunctionType
OP = mybir.AluOpType

B, S, D, H, K, MLP, ZD = 8, 2048, 1024, 16, 256, 4096, 1024
DH = D // H      # 64
P = 128
SC = S // P      # 16
DC = D // P      # 8
NG = 4
GS = 512
MC = MLP // P    # 32
KC = K // P      # 2
EPS = 1e-6

W2D = [("wq", D, D), ("wk", D, D), ("wv", D, D), ("wo", D, D),
       ("Ew", S, K), ("Fw", S, K),
       ("h1w", ZD, D), ("g1w", D, D), ("be1w", D, D),
       ("h2w", ZD, D), ("g2w", D, D), ("be2w", D, D),
       ("m1w", D, MLP), ("m2w", MLP, D)]
W1D = [("bq", D), ("bk", D), ("bv", D), ("bo", D), ("Eb", K), ("Fb", K),
       ("h1b", D), ("g1b", D), ("be1b", D), ("h2b", D), ("g2b", D), ("be2b", D),
       ("m1b", MLP), ("m2b", D)]

_cache = {}


def build():
    if "nc" in _cache:
        return _cache["nc"]
    nc = bacc.Bacc("TRN2", target_bir_lowering=False, debug=False, num_devices=8)
    ap = {}
    ap["x"] = nc.dram_tensor("x", [S, D], f32, kind="ExternalInput").ap()
    ap["z"] = nc.dram_tensor("z", [1, ZD], f32, kind="ExternalInput").ap()
    for nm, a, b in W2D:
        ap[nm] = nc.dram_tensor(nm, [a, b], f32, kind="ExternalInput").ap()
    for nm, a in W1D:
        ap[nm] = nc.dram_tensor(nm, [a], f32, kind="ExternalInput").ap()
    out = nc.dram_tensor("out", [S, D], f32, kind="ExternalOutput").ap()
    with tile.TileContext(nc, trace_sim=False) as tc:
        _emit(nc, tc, ap, out)
    nc.compile()
    _cache["nc"] = nc
    return nc


def _emit(nc, tc, ap, out):
    ctx = contextlib.ExitStack()
    with ctx:
        # ---------- whole-kernel pools ----------
        const = ctx.enter_context(tc.tile_pool(name="const", bufs=1))
        cols = ctx.enter_context(tc.tile_pool(name="cols", bufs=1))
        rows = ctx.enter_context(tc.tile_pool(name="rows", bufs=1))

        ident_f = const.tile([P, P], f32, tag="ident_f", name="ident_f")
        make_identity(nc, ident_f)
        ident_h = const.tile([P, P], f16, tag="ident_h", name="ident_h")
        nc.vector.tensor_copy(ident_h[:], ident_f[:])
        eps_t = const.tile([P, 1], f32, tag="eps", name="eps")
        nc.vector.memset(eps_t[:], EPS)
        ones1_h = const.tile([1, P], f16, tag="ones1_h", name="ones1_h")
        nc.vector.memset(ones1_h[:], 1.0)
        onescol_h = const.tile([P, 1], f16, tag="onescol_h", name="onescol_h")
        nc.vector.memset(onescol_h[:], 1.0)

        def col_load(name, n):
            t = cols.tile([P, n], f32, tag=f"cols_{name}", name=f"cols_{name}")
            for j in range(n):
                nc.sync.dma_start(t[:, j:j + 1], ap[name][ds(P * j, P)])
            return t

        def row_to_cols(tag, row_f, n=DC):
            cf = cols.tile([P, n], f32, tag=f"c_{tag}", name=f"c_{tag}")
            for j in range(n):
                nc.sync.dma_start(cf[:, j:j + 1], row_f[0:1, ds(P * j, P)])
            return cf

        def bcast_rows(tag, row_f, n, psp, pool, rpool):
            row_h = rpool.tile([1, n], f16, tag=f"rr_{tag}", name=f"rr_{tag}",
                               bufs=1)
            nc.vector.tensor_copy(row_h[:], row_f[0:1, 0:n])
            t = pool.tile([P, n], f32, tag=f"bc_{tag}", name=f"bc_{tag}")
            for h in range(0, n, GS):
                w = min(GS, n - h)
                pt = psp.tile([P, GS], f32, tag="bc_ps", name="bc_ps")
                nc.tensor.matmul(pt[:, 0:w], ones1_h[:], row_h[0:1, h:h + w],
                                 start=True, stop=True)
                nc.scalar.copy(t[:, h:h + w], pt[:, 0:w])
            return t

        # small, cheap DMAs first
        zc_f = cols.tile([P, DC], f32, tag="zc_f", name="zc_f")
        for j in range(DC):
            nc.sync.dma_start(zc_f[:, j:j + 1], ap["z"][0:1, ds(P * j, P)])
        zc_h = cols.tile([P, DC], f16, tag="zc_h", name="zc_h")
        nc.vector.tensor_copy(zc_h[:], zc_f[:])
        mv_t = cols.tile([P, SC, 2], f32, tag="mv", name="mv")

        # cross-phase stacks.  Right side LIFO (bottom->top):
        # aoT, qT, kv, pef, wkv.  Left: s_b{arow,x1T,wq,x1n} then
        # bc4, at, resw, C2-staging, MLP pools.
        s_resw = contextlib.ExitStack()
        s_b = contextlib.ExitStack()
        s_x1n = contextlib.ExitStack()
        s_wkv = contextlib.ExitStack()
        s_c2r = contextlib.ExitStack()    # right: aoT + qT + kv
        s_pef = contextlib.ExitStack()

        aoT_p = s_c2r.enter_context(tc.tile_pool(name="aoT", bufs=1,
                                                 side="right"))
        qT_p = s_c2r.enter_context(tc.tile_pool(name="qT", bufs=1,
                                                side="right"))
        kv_sb = s_c2r.enter_context(tc.tile_pool(name="kv_sb", bufs=1,
                                                 side="right"))
        pef_sb = s_pef.enter_context(
            tc.tile_pool(name="pef_sb", bufs=1, side="right"))
        arow = s_b.enter_context(tc.tile_pool(name="arow", bufs=1))
        x1T_p = s_b.enter_context(tc.tile_pool(name="x1T", bufs=1))
        wq_p = s_b.enter_context(tc.tile_pool(name="wq", bufs=1))
        vec_ctx = contextlib.ExitStack()
        vsb = vec_ctx.enter_context(tc.tile_pool(name="vec_sb", bufs=4))
        vps = vec_ctx.enter_context(
            tc.tile_pool(name="vec_ps", bufs=2, space="PSUM"))
        x1n_p = s_x1n.enter_context(tc.tile_pool(name="x1nat", bufs=1))

        qT = [[qT_p.tile([P, GS], f16, tag=f"qT_{j}_{g}", name=f"qT_{j}_{g}")
               for g in range(NG)] for j in range(DC)]
        pefEF = [pef_sb.tile([P, 2 * K], f16, tag=f"pef{j}", name=f"pef{j}")
                 for j in range(DC)]
        x1n = []

        def a_row_load(name, n):
            t = arow.tile([1, n], f32, tag="mrow", name=f"row_{name}", bufs=2)
            nc.sync.dma_start(t[:], ap[name][0:n])
            return t

        def a_half_load(name, h):
            t = arow.tile([1, GS], f32, tag="biash", name=f"rh_{name}{h}",
                          bufs=2)
            nc.sync.dma_start(t[:], ap[name][ds(GS * h, GS)])
            return t

        h1_row = arow.tile([1, D], f32, tag="hrow", name="h1", bufs=1)
        h2_row = arow.tile([1, D], f32, tag="hrow", name="h2", bufs=1)
        sc1_row = arow.tile([1, D], f32, tag="srow", name="sc1", bufs=2)
        of1_row = arow.tile([1, D], f32, tag="srow", name="of1", bufs=2)
        sc2_row = arow.tile([1, D], f32, tag="srow", name="sc2", bufs=2)
        of2_row = arow.tile([1, D], f32, tag="srow", name="of2", bufs=2)

        # ===== interleaved: LN1 groups + conditioning GEMV layers =====
        ln_ctx = contextlib.ExitStack()
        ln_sb = ln_ctx.enter_context(tc.tile_pool(name="ln1_sb", bufs=2))

        def ln1_group(g):
            for ii in range(4):
                i = 4 * g + ii
                xt = ln_sb.tile([P, D], f32, tag="ln_in", name="ln_in",
                                bufs=3)
                nc.sync.dma_start(xt[:], ap["x"][ds(P * i, P), :])
                st = ln_sb.tile([P, 2, 6], f32, tag="ln_st", name="ln_st")
                nc.vector.bn_stats(st[:, 0, :], xt[:, 0:GS])
                nc.vector.bn_stats(st[:, 1, :], xt[:, GS:D])
                mv = ln_sb.tile([P, 2], f32, tag="ln_mv", name="ln_mv")
                nc.vector.bn_aggr(mv[:], st[:])
                sd = ln_sb.tile([P, 1], f32, tag="ln_sd", name="ln_sd")
                nc.scalar.activation(sd[:], mv[:, 1:2], AF.Sqrt,
                                     bias=eps_t[:])
                rstd = ln_sb.tile([P, 1], f32, tag="ln_rstd", name="ln_rstd")
                nc.vector.reciprocal_approx_fast(rstd[:], sd[:])
                nmr = ln_sb.tile([P, 1], f32, tag="ln_nmr", name="ln_nmr")
                nc.vector.tensor_scalar(nmr[:], mv[:, 0:1], rstd[:],
                                        -1.0, OP.mult, OP.mult)
                x1t = x1n_p.tile([P, D], f16, tag=f"nat{i}", name=f"nat{i}")
                nc.scalar.activation(x1t[:], xt[:], AF.Identity,
                                     bias=nmr[:], scale=rstd[:])
                x1n.append(x1t)

        def vec_layer(wname, lhs_cols, bias_name, act, out_row, cast16):
            pts = [vps.tile([1, GS], f32, tag=f"vps{h}", name=f"vps{h}",
                            bufs=1) for h in range(2)]
            for j in range(DC):
                for h in range(2):
                    wt = vsb.tile([P, GS], f32, tag="vw_f", name="vw_f")
                    nc.sync.dma_start(wt[:],
                                      ap[wname][ds(P * j, P), ds(GS * h, GS)])
                    wh = vsb.tile([P, GS], f16, tag="vw_h", name="vw_h")
                    if (2 * j + h) % 2 == 0:
                        nc.vector.tensor_copy(wh[:], wt[:])
                    else:
                        nc.scalar.copy(wh[:], wt[:])
                    nc.tensor.matmul(pts[h][:], lhs_cols[:, j:j + 1],
                                     wh[:],
                                     start=(j == 0), stop=(j == DC - 1))
            for h in range(2):
                bias_h = a_half_load(bias_name, h)
                pre = arow.tile([1, GS], f32, tag=f"vpre{h}",
                                name=f"vpre{h}", bufs=1)
                nc.vector.tensor_add(pre[:], bias_h[:], pts[h][:])
                if act is None:
                    nc.vector.tensor_copy(out_row[0:1, ds(GS * h, GS)], pre[:])
                else:
                    nc.scalar.activation(out_row[0:1, ds(GS * h, GS)],
                                         pre[:], act)

        # g0 LN, then h1 chain, interleaving LN groups between layers
        ln1_group(0)
        vec_layer("h1w", zc_h, "h1b", AF.Silu, h1_row, False)
        h1_c = row_to_cols("h1", h1_row)
        h1_ch = cols.tile([P, DC], f16, tag="h1_ch", name="h1_ch")
        nc.vector.tensor_copy(h1_ch[:], h1_c[:])
        vec_layer("g1w", h1_ch, "g1b", None, sc1_row, True)
        vec_layer("be1w", h1_ch, "be1b", None, of1_row, True)
        ln1_group(1)
        rec1_row = arow.tile([1, D], f32, tag="mrow", name="rec1", bufs=2)
        nc.vector.reciprocal_approx_fast(rec1_row[:], sc1_row[:])
        o1s_row = arow.tile([1, D], f32, tag="mrow", name="o1s", bufs=2)
        nc.vector.tensor_mul(o1s_row[:], of1_row[:], rec1_row[:])
        s1_c = row_to_cols("s1", sc1_row)
        o1s_c = row_to_cols("o1s", o1s_row)
        o1s_ch = cols.tile([P, DC], f16, tag="o1s_ch", name="o1s_ch")
        nc.vector.tensor_copy(o1s_ch[:], o1s_c[:])
        ln1_group(2)
        ln1_group(3)
        ln_ctx.close()
        # wq resident with fused scale1 row-scaling (via vec staging)
        wq_r = []
        for j in range(DC):
            wr = wq_p.tile([P, D], f16, tag=f"wq_r{j}", name=f"wq_r{j}")
            for h in range(2):
                wt = vsb.tile([P, GS], f32, tag="vw_f", name="vw_f")
                nc.sync.dma_start(wt[:],
                                  ap["wq"][ds(P * j, P), ds(GS * h, GS)])
                nc.vector.tensor_scalar(wr[:, ds(GS * h, GS)], wt[:],
                                        s1_c[:, j:j + 1], None, OP.mult)
            wq_r.append(wr)

        # ---- PEF (two j-half passes, Ew|Fw re-streamed) + colsums ----
        cs_row = rows.tile([1, 2 * K], f32, tag="cs", name="cs")
        cs2_b = rows.tile([P, 2 * K], f32, tag="cs2b", name="cs2b")
        with (
            tc.tile_pool(name="ef_st", bufs=4) as efst,
            tc.tile_pool(name="pef_ps", bufs=1, space="PSUM") as pfps,
            tc.tile_pool(name="cs_ps", bufs=1, space="PSUM") as csps,
        ):
            cs_ps = csps.tile([1, 2 * K], f32, tag="cs_ps", name="cs_ps")
            for jh in range(2):
                pef_ps = [pfps.tile([P, 2 * K], f32, tag=f"pefp{jj}",
                                    name=f"pefp{jj}") for jj in range(4)]
                for i in range(SC):
                    ff = efst.tile([P, 2 * K], f32, tag="ef_f", name="ef_f")
                    nc.sync.dma_start(ff[:, 0:K], ap["Ew"][ds(P * i, P), :])
                    nc.sync.dma_start(ff[:, K:2 * K],
                                      ap["Fw"][ds(P * i, P), :])
                    fr = efst.tile([P, 2 * K], f16, tag="ef_r", name="ef_r",
                                   bufs=3)
                    nc.scalar.copy(fr[:], ff[:])
                    if jh == 0:
                        nc.tensor.matmul(cs_ps[:], onescol_h[:], fr[:],
                                         start=(i == 0), stop=(i == SC - 1))
                    for jj in range(4):
                        j = 4 * jh + jj
                        nc.tensor.matmul(pef_ps[jj][:],
                                         x1n[i][:, ds(P * j, P)], fr[:],
                                         start=(i == 0), stop=(i == SC - 1))
                if jh == 0:
                    nc.vector.tensor_copy(cs_row[:], cs_ps[:])
                    cs_row_h = arow.tile([1, 2 * K], f16, tag="cs_h",
                                         name="cs_h")
                    nc.vector.tensor_copy(cs_row_h[:], cs_row[:])
                    with tc.tile_pool(name="csb_ps", bufs=1,
                                      space="PSUM") as cbps:
                        cpt = cbps.tile([P, 2 * K], f32, tag="cs_bc",
                                        name="cs_bc")
                        nc.tensor.matmul(cpt[:], ones1_h[:], cs_row_h[:],
                                         start=True, stop=True)
                        nc.scalar.copy(cs2_b[:], cpt[:])
                for jj in range(4):
                    j = 4 * jh + jj
                    nc.vector.scalar_tensor_tensor(
                        pefEF[j][:], cs2_b[:], o1s_c[:, j:j + 1],
                        pef_ps[jj][:], OP.mult, OP.add)

        # ---- qT: transposes + bias GEMV + matmuls ----
        bq_eff_c = cols.tile([P, DC], f32, tag="bqe_c", name="bqe_c")
        with tc.tile_pool(name="q_ps", bufs=2, space="PSUM") as qps:
            bq_eff_row = arow.tile([1, D], f32, tag="mrow", name="bqe",
                                   bufs=2)
            gps = [qps.tile([1, GS], f32, tag=f"gv{h}", name=f"gv{h}",
                            bufs=1) for h in range(2)]
            for j in range(DC):
                for h in range(2):
                    nc.tensor.matmul(gps[h][:], o1s_ch[:, j:j + 1],
                                     wq_r[j][:, ds(GS * h, GS)],
                                     start=(j == 0), stop=(j == DC - 1))
            bq_row = a_row_load("bq", D)
            for h in range(2):
                nc.vector.tensor_add(bq_eff_row[0:1, ds(GS * h, GS)],
                                     bq_row[0:1, ds(GS * h, GS)], gps[h][:])
            for j in range(DC):
                nc.sync.dma_start(bq_eff_c[:, j:j + 1],
                                  bq_eff_row[0:1, ds(P * j, P)])
            with tc.tile_pool(name="tp1_ps2", bufs=2,
                              space="PSUM") as tpps:
                for g in range(NG):
                    x1T_g = [x1T_p.tile([P, GS], f16, tag=f"x1T_{j}",
                                        name=f"x1T_{j}", bufs=2)
                             for j in range(DC)]
                    for ii in range(4):
                        x1t = x1n[4 * g + ii]
                        for j in range(DC):
                            pt = tpps.tile([P, P], f16, tag="tp_ps",
                                           name="tp_ps")
                            nc.tensor.transpose(pt[:], x1t[:, ds(P * j, P)],
                                                ident_h[:])
                            if j % 2 == 0:
                                nc.vector.tensor_copy(
                                    x1T_g[j][:, ds(P * ii, P)], pt[:])
                            else:
                                nc.scalar.copy(
                                    x1T_g[j][:, ds(P * ii, P)], pt[:])
                    for jo in range(DC):
                        pt = qps.tile([P, GS], f32, tag="q_ps", name="q_ps")
                        for j in range(DC):
                            nc.tensor.matmul(pt[:], wq_r[j][:, ds(P * jo, P)],
                                             x1T_g[j][:],
                                             start=(j == 0),
                                             stop=(j == DC - 1))
                        nc.scalar.activation(qT[jo][g][:], pt[:], AF.Identity,
                                             bias=bq_eff_c[:, jo:jo + 1])
        s_x1n.close()
        # late conditioning chain (scale2/offset2) overlaps qT on DMA
        vec_layer("h2w", zc_h, "h2b", AF.Silu, h2_row, False)
        h2_c = row_to_cols("h2", h2_row)
        h2_ch = cols.tile([P, DC], f16, tag="h2_ch", name="h2_ch")
        nc.vector.tensor_copy(h2_ch[:], h2_c[:])
        vec_layer("g2w", h2_ch, "g2b", None, sc2_row, True)
        vec_layer("be2w", h2_ch, "be2b", None, of2_row, True)
        rec2_row = arow.tile([1, D], f32, tag="mrow", name="rec2", bufs=2)
        nc.vector.reciprocal_approx_fast(rec2_row[:], sc2_row[:])
        o2s_row = arow.tile([1, D], f32, tag="mrow", name="o2s", bufs=2)
        nc.vector.tensor_mul(o2s_row[:], of2_row[:], rec2_row[:])
        s2_c = row_to_cols("s2", sc2_row)
        o2s_c = row_to_cols("o2s", o2s_row)
        vec_ctx.close()
        s_b.close()   # frees arow, x1T, wq

        # wk/wv residents (right stack top, freed after KV)
        wkv_p = s_wkv.enter_context(tc.tile_pool(name="wkv", bufs=1,
                                                 side="right"))
        wk_r, wv_r = [], []
        with tc.tile_pool(name="wkv_st", bufs=3) as wkst:
            for nm, lst in (("wk", wk_r), ("wv", wv_r)):
                for j in range(DC):
                    wt = wkst.tile([P, D], f32, tag="wkv_f", name="wkv_f")
                    nc.sync.dma_start(wt[:], ap[nm][ds(P * j, P), :])
                    wr = wkv_p.tile([P, D], f16, tag=f"{nm}_r{j}",
                                    name=f"{nm}_r{j}")
                    nc.vector.tensor_scalar(wr[:], wt[:], s1_c[:, j:j + 1],
                                            None, OP.mult)
                    lst.append(wr)

        # ===== phase KV: kpT + vpe (with fused ones cols) =====
        kpT = [kv_sb.tile([P, K], f16, tag=f"kpT{j}", name=f"kpT{j}")
               for j in range(DC)]
        vpe = [kv_sb.tile([P, P * H], f16, tag=f"vpe{c}", name=f"vpe{c}")
               for c in range(KC)]
        with (
            tc.tile_pool(name="kv_bias", bufs=1) as kvb,
            tc.tile_pool(name="kv_tmp", bufs=2) as kvt,
            tc.tile_pool(name="kv_ps", bufs=2, space="PSUM") as kvps,
        ):
            def kv_row_load(name, n):
                t = kvb.tile([1, n], f32, tag=f"kvrow_{name}",
                             name=f"kvrow_{name}")
                nc.sync.dma_start(t[:], ap[name][0:n])
                return t

            bk_c = col_load("bk", DC)
            Fb_c = col_load("Fb", KC)
            bv_row = kv_row_load("bv", D)
            Eb_row = kv_row_load("Eb", K)
            with tc.tile_pool(name="kv_bc_ps", bufs=2, space="PSUM") as kbps:
                bv_b = bcast_rows("bv", bv_row, D, kbps, kvb, kvt)
                Eb_b = bcast_rows("Eb", Eb_row, K, kbps, kvb, kvt)
            csF_c = kvb.tile([P, KC], f32, tag="csF_c", name="csF_c")
            for c in range(KC):
                nc.sync.dma_start(csF_c[:, c:c + 1],
                                  cs_row[0:1, ds(K + P * c, P)])
            kp_bias = []
            for j in range(DC):
                bt = kvb.tile([P, K], f32, tag=f"kpb{j}", name=f"kpb{j}")
                nc.vector.tensor_scalar(bt[:], cs2_b[:, 0:K], bk_c[:, j:j + 1],
                                        None, OP.mult)
                nc.vector.tensor_add(bt[:], bt[:], Eb_b[:])
                kp_bias.append(bt)
            vp_bias = []
            for c in range(KC):
                bt = kvb.tile([P, D], f32, tag=f"vpb{c}", name=f"vpb{c}")
                nc.vector.tensor_scalar(bt[:], bv_b[:], csF_c[:, c:c + 1],
                                        Fb_c[:, c:c + 1], OP.mult, OP.add)
                vp_bias.append(bt)

            for c in range(KC):
                for h in range(H):
                    nc.vector.memset(vpe[c][:, ds(P * h + DH, DH)], 1.0)
            for jo in range(DC):
                pt = kvps.tile([P, K], f32, tag="kp_ps", name="kp_ps")
                for j in range(DC):
                    nc.tensor.matmul(pt[:], wk_r[j][:, ds(P * jo, P)],
                                     pefEF[j][:, 0:K],
                                     start=(j == 0), stop=(j == DC - 1))
                nc.vector.tensor_add(kpT[jo][:], kp_bias[jo][:], pt[:])
            for hf in range(2):
                for c in range(KC):
                    pt = kvps.tile([P, GS], f32, tag="vp_ps", name="vp_ps")
                    for j in range(DC):
                        nc.tensor.matmul(pt[:],
                                         pefEF[j][:, ds(K + P * c, P)],
                                         wv_r[j][:, ds(GS * hf, GS)],
                                         start=(j == 0), stop=(j == DC - 1))
                    tmp = kvt.tile([P, GS], f32, tag="vp_tmp", name="vp_tmp")
                    nc.vector.tensor_add(tmp[:],
                                         vp_bias[c][:, ds(GS * hf, GS)], pt[:])
                    for hh in range(8):
                        h = 8 * hf + hh
                        if hh % 2 == 0:
                            nc.vector.tensor_copy(vpe[c][:, ds(P * h, DH)],
                                                  tmp[:, ds(DH * hh, DH)])
                        else:
                            nc.scalar.copy(vpe[c][:, ds(P * h, DH)],
                                           tmp[:, ds(DH * hh, DH)])
        s_wkv.close()
        s_pef.close()

        # left-side post-B long-lived pools + wo resident
        bc4 = ctx.enter_context(tc.tile_pool(name="bc4", bufs=1))
        at_p = ctx.enter_context(tc.tile_pool(name="at", bufs=1))
        resw = s_resw.enter_context(tc.tile_pool(name="resw", bufs=1))
        wo_r = []
        with tc.tile_pool(name="wo_st", bufs=3) as wost:
            with tc.tile_pool(name="wo_bc_ps", bufs=2, space="PSUM") as wbps:
                bo_row = wost.tile([1, D], f32, tag="bo_row", name="bo_row",
                                   bufs=1)
                nc.sync.dma_start(bo_row[:], ap["bo"][0:D])
                m2b_row = wost.tile([1, D], f32, tag="m2b_row",
                                    name="m2b_row", bufs=1)
                nc.sync.dma_start(m2b_row[:], ap["m2b"][0:D])
                bo_b = bcast_rows("bo", bo_row, D, wbps, bc4, wost)
                m2b_b = bcast_rows("m2b", m2b_row, D, wbps, bc4, wost)
            for j in range(DC):
                wt = wost.tile([P, D], f32, tag="wo_f", name="wo_f")
                nc.sync.dma_start(wt[:], ap["wo"][ds(P * j, P), :])
                wr = resw.tile([P, D], f16, tag=f"wo_r{j}", name=f"wo_r{j}")
                if j % 2 == 0:
                    nc.scalar.copy(wr[:], wt[:])
                else:
                    nc.vector.tensor_copy(wr[:], wt[:])
                wo_r.append(wr)

        # ===== C2 + C3 fused per token group =====
        aoT = [[aoT_p.tile([P, GS], f16, tag=f"aoT_{j}_{g}",
                           name=f"aoT_{j}_{g}")
                for g in range(NG)] for j in range(DC)]
        at = [at_p.tile([P, D], f32, tag=f"at{i}", name=f"at{i}")
              for i in range(SC)]
        with (
            tc.tile_pool(name="at_sb", bufs=8) as atsb,
            tc.tile_pool(name="den_sb", bufs=2) as densb,
            tc.tile_pool(name="c3_sb", bufs=3) as c3sb,
            tc.tile_pool(name="sc_ps", bufs=3, space="PSUM") as scps,
            tc.tile_pool(name="av_ps", bufs=3, space="PSUM") as avps,
            tc.tile_pool(name="wo_ps", bufs=2, space="PSUM") as wops,
        ):
            def emit_scores_pair(g, p):
                exps = [[None, None], [None, None]]
                for c in range(KC):
                    for e in range(2):
                        r0 = 64 * e
                        spt = scps.tile([P, GS], f32, tag="sc", name="sc")
                        nc.tensor.matmul(spt[:],
                                         kpT[p][r0:r0 + 64, ds(P * c, P)],
                                         qT[p][g][r0:r0 + 64, :],
                                         start=True, stop=True)
                        et = atsb.tile([P, GS], f16, tag="exp", name="exp")
                        nc.scalar.activation(et[:], spt[:], AF.Exp,
                                             scale=0.125)
                        exps[e][c] = et
                return exps

            def emit_c3_chunk(g, ii):
                i = 4 * g + ii
                xt2 = c3sb.tile([P, D], f32, tag="res_x", name="res_x",
                                bufs=3)
                nc.sync.dma_start(xt2[:], ap["x"][ds(P * i, P), :])
                for half in range(2):
                    pt = wops.tile([P, GS], f32, tag="wo_ps", name="wo_ps")
                    for j in range(DC):
                        nc.tensor.matmul(pt[:],
                                         aoT[j][g][:, ds(P * ii, P)],
                                         wo_r[j][:, ds(GS * half, GS)],
                                         start=(j == 0), stop=(j == DC - 1))
                    tm = c3sb.tile([P, GS], f32, tag="wo_tmp",
                                   name="wo_tmp", bufs=2)
                    nc.vector.tensor_add(tm[:], bo_b[:, ds(GS * half, GS)],
                                         pt[:])
                    nc.gpsimd.tensor_add(at[i][:, ds(GS * half, GS)],
                                         tm[:], xt2[:, ds(GS * half, GS)])
                st = c3sb.tile([P, 2, 6], f32, tag="c3_st", name="c3_st")
                nc.vector.bn_stats(st[:, 0, :], at[i][:, 0:GS])
                nc.vector.bn_stats(st[:, 1, :], at[i][:, GS:D])
                nc.vector.bn_aggr(mv_t[:, i, :], st[:])

            for g in range(NG):
                exps = emit_scores_pair(g, 0)
                for p in range(8):
                    nxt = emit_scores_pair(g, p + 1) if p < 7 else None
                    apts = []
                    for e in range(2):
                        h = 2 * p + e
                        apt = avps.tile([P, GS], f32, tag="av", name="av")
                        for c in range(KC):
                            nc.tensor.matmul(apt[:], vpe[c][:, ds(P * h, P)],
                                             exps[e][c][:],
                                             start=(c == 0),
                                             stop=(c == KC - 1))
                        apts.append(apt)
                    den = densb.tile([64, 2 * GS], f32, tag="den", name="den")
                    nc.scalar.copy(den[:, 0:GS], apts[0][64:128, :])
                    nc.scalar.copy(den[:, GS:2 * GS], apts[1][64:128, :])
                    rec = densb.tile([64, 2 * GS], f32, tag="rec", name="rec",
                                     bufs=1)
                    nc.vector.reciprocal_approx_fast(rec[:], den[:])
                    nc.vector.tensor_mul(aoT[p][g][0:64, :],
                                         apts[0][0:64, :], rec[:, 0:GS])
                    nc.vector.tensor_mul(aoT[p][g][64:128, :],
                                         apts[1][0:64, :], rec[:, GS:2 * GS])
                    if g > 0 and p % 2 == 1:
                        emit_c3_chunk(g - 1, p // 2)
                    exps = nxt
            for ii in range(4):
                emit_c3_chunk(NG - 1, ii)
        s_resw.close()
        s_c2r.close()

        # ===== LN2 + MLP, two supergroups of 1024 tokens =====
        m1b_c = col_load("m1b", MC)
        for sg in range(2):
            sctx = contextlib.ExitStack()
            with sctx:
                x2T_p = sctx.enter_context(tc.tile_pool(name="x2T", bufs=1))
                hm_p = sctx.enter_context(tc.tile_pool(name="hm", bufs=1))
                x2T = [x2T_p.tile([P, 2 * GS], f16, tag=f"x2T_{j}",
                                  name=f"x2T_{j}") for j in range(DC)]
                hm = [hm_p.tile([P, 2 * GS], f16, tag=f"hm{m}", name=f"hm{m}")
                      for m in range(MC)]
                with (
                    tc.tile_pool(name="ln2_sb", bufs=2) as ln2sb,
                    tc.tile_pool(name="tp2_ps", bufs=2, space="PSUM") as tp2ps,
                ):
                    for gg in range(2):
                        g = 2 * sg + gg
                        sd4 = ln2sb.tile([P, 4], f32, tag="l2sd", name="l2sd")
                        for ii in range(4):
                            i = 4 * g + ii
                            nc.scalar.activation(sd4[:, ii:ii + 1],
                                                 mv_t[:, i, 1:2],
                                                 AF.Sqrt, bias=eps_t[:])
                        rstd4 = ln2sb.tile([P, 4], f32, tag="l2rs",
                                           name="l2rs")
                        nc.vector.reciprocal_approx_fast(rstd4[:], sd4[:])
                        for ii in range(4):
                            i = 4 * g + ii
                            rstd = rstd4[:, ii:ii + 1]
                            nmr = ln2sb.tile([P, 1], f32, tag="l2nm",
                                             name="l2nm")
                            nc.vector.tensor_scalar(nmr[:], mv_t[:, i, 0:1],
                                                    rstd, -1.0,
                                                    OP.mult, OP.mult)
                            x2t = ln2sb.tile([P, D], f16, tag="x2nat",
                                             name="x2nat")
                            nc.scalar.activation(x2t[:], at[i][:],
                                                 AF.Identity,
                                                 bias=nmr[:], scale=rstd)
                            tl = GS * gg + P * ii
                            for j in range(DC):
                                pt = tp2ps.tile([P, P], f16, tag="tp2",
                                                name="tp2")
                                nc.tensor.transpose(pt[:],
                                                    x2t[:, ds(P * j, P)],
                                                    ident_h[:])
                                nc.scalar.activation(x2T[j][:, ds(tl, P)],
                                                     pt[:], AF.Identity,
                                                     bias=o2s_c[:, j:j + 1])
                with (
                    tc.tile_pool(name="m1_st", bufs=6) as m1st,
                    tc.tile_pool(name="m1h_st", bufs=8) as m1hst,
                    tc.tile_pool(name="m1_ps", bufs=2, space="PSUM") as m1ps,
                ):
                    for mq in range(8):
                        w1h = []
                        for j in range(DC):
                            wf = m1st.tile([P, GS], f32, tag="m1f",
                                           name="m1f")
                            nc.sync.dma_start(
                                wf[:],
                                ap["m1w"][ds(P * j, P), ds(GS * mq, GS)])
                            wh = m1hst.tile([P, GS], f16, tag="m1h",
                                            name="m1h")
                            if j % 2 == 0:
                                nc.vector.tensor_scalar(
                                    wh[:], wf[:], s2_c[:, j:j + 1], None,
                                    OP.mult)
                            else:
                                nc.scalar.activation(
                                    wh[:], wf[:], AF.Copy,
                                    scale=s2_c[:, j:j + 1])
                            w1h.append(wh)
                        for mm in range(4):
                            m = 4 * mq + mm
                            for g2 in range(2):
                                pt = m1ps.tile([P, GS], f32, tag="m1p",
                                               name="m1p")
                                for j in range(DC):
                                    nc.tensor.matmul(
                                        pt[:], w1h[j][:, ds(P * mm, P)],
                                        x2T[j][:, ds(GS * g2, GS)],
                                        start=(j == 0), stop=(j == DC - 1))
                                nc.scalar.activation(
                                    hm[m][:, ds(GS * g2, GS)], pt[:],
                                    AF.Gelu, bias=m1b_c[:, m:m + 1])
                with (
                    tc.tile_pool(name="m2_st", bufs=6) as m2st,
                    tc.tile_pool(name="m2h_st", bufs=5) as m2hst,
                    tc.tile_pool(name="e_sb", bufs=4) as esb,
                    tc.tile_pool(name="m2_ps", bufs=1, space="PSUM") as m2ps,
                ):
                    for half in range(2):
                        m2p = [m2ps.tile([P, GS], f32, tag=f"m2p{ss}",
                                         name=f"m2p{ss}") for ss in range(8)]
                        for m in range(MC):
                            wf = m2st.tile([P, GS], f32, tag="m2f",
                                           name="m2f")
                            nc.sync.dma_start(
                                wf[:],
                                ap["m2w"][ds(P * m, P), ds(GS * half, GS)])
                            wh = m2hst.tile([P, GS], f16, tag="m2h",
                                            name="m2h")
                            if m % 2 == 0:
                                nc.vector.tensor_copy(wh[:], wf[:])
                            else:
                                nc.scalar.copy(wh[:], wf[:])
                            for ss in range(8):
                                nc.tensor.matmul(
                                    m2p[ss][:], hm[m][:, ds(P * ss, P)],
                                    wh[:],
                                    start=(m == 0), stop=(m == MC - 1))
                        for ss in range(8):
                            i = 8 * sg + ss
                            tm = esb.tile([P, GS], f32, tag="e_tmp",
                                          name="e_tmp")
                            nc.vector.tensor_add(
                                tm[:], m2b_b[:, ds(GS * half, GS)],
                                m2p[ss][:])
                            ot = esb.tile([P, GS], f32, tag="e_out",
                                          name="e_out")
                            nc.gpsimd.tensor_add(
                                ot[:], tm[:], at[i][:, ds(GS * half, GS)])
                            nc.sync.dma_start(
                                out[ds(P * i, P), ds(GS * half, GS)], ot[:])


def kernel(**inputs):
    nc = build()
    x = np.ascontiguousarray(inputs["x"], dtype=np.float32)
    z = np.ascontiguousarray(inputs["z"], dtype=np.float32)
    base = {}
    for nm, _, _ in W2D:
        base[nm] = np.ascontiguousarray(inputs[nm], dtype=np.float32)
    for nm, _ in W1D:
        base[nm] = np.ascontiguousarray(inputs[nm], dtype=np.float32)
    in_maps = []
    for c in range(B):
        m = dict(base)
        m["x"] = x[c]
        m["z"] = z[c:c + 1]
        in_maps.append(m)
    res = run_bass_kernel_spmd(nc, in_maps, list(range(B)))
    _cache["last"] = res
    return np.stack([res.results[c]["out"] for c in range(B)], axis=0)


# revision 29
# speedup vs baseline: 1.0398x; 1.0398x over previous
"""DiT block (Linformer attention + adaLN + MLP) on 8 TRN2 NeuronCores.

Sharding: data-parallel over batch (B=8 -> one batch element per core).

v2 design notes:
 - adaLN conditioning folded into weights: wq/wk/wv rows scaled by scale1
   during their f32->f16 resident cast (tensor_scalar = same cost as the
   plain cast), m1w rows scaled by scale2 during its streamed cast.
   LayerNorm emits pure xn in f16; offsets enter via per-partition biases:
   qT gets (wq^T o1 + bq) columns (one PE GEMV), P_EF gets a rank-1
   (o1/s1) x colsum(E|F) correction fused into its PSUM evacuation
   (scalar_tensor_tensor), x2T gets (o2/s2) columns at transpose-evac.
 - Ew|Fw streamed once; colsums accumulate in the same pass; the f16 copy
   is kept in SBUF for the second PEF j-half pass.
 - Attention: the A@V stationary operand carries V in cols 0:64 and ONES
   in cols 64:128 of each head block, so softmax denominators land on PSUM
   partitions 64:128 at no extra PE cost.  Evac denominators (scalar),
   reciprocal on SBUF (DVE), multiply PSUM x SBUF -> normalized aoT f16
   (DVE).  No PE broadcasts, no single-partition row ops.
 - wo + residual + LN2 bn_stats fused per token group right after that
   group's attention pairs; attention residual kept in SBUF (no DRAM
   round-trip).
 - MLP in 2 supergroups of 1024 tokens: m1w/m2w streamed fp32 and cast
   inline to f16 (m1 cast fused with the scale2 fold), gelu emits f16 hm,
   m2 accumulates 8 PSUM banks per D-half, residual added from SBUF.
 - Emission interleaves LN1 groups with the conditioning GEMV layers so
   the DMA queues serve x and h1w/g1w before the bulk weight streams.
"""
import contextlib

import numpy as np

import concourse.bass as bass
import concourse.mybir as mybir
import concourse.tile as tile
from concourse import bacc
from concourse.bass import ds
from concourse.bass_utils import run_bass_kernel_spmd
from concourse.masks import make_identity

f32 = mybir.dt.float32
f16 = mybir.dt.float16
AF = mybir.ActivationFunctionType
OP = mybir.AluOpType

B, S, D, H, K, MLP, ZD = 8, 2048, 1024, 16, 256, 4096, 1024
DH = D // H      # 64
P = 128
SC = S // P      # 16
DC = D // P      # 8
NG = 4
GS = 512
MC = MLP // P    # 32
KC = K // P      # 2
EPS = 1e-6

W2D = [("wq", D, D), ("wk", D, D), ("wv", D, D), ("wo", D, D),
       ("Ew", S, K), ("Fw", S, K),
       ("h1w", ZD, D), ("g1w", D, D), ("be1w", D, D),
       ("h2w", ZD, D), ("g2w", D, D), ("be2w", D, D),
       ("m1w", D, MLP), ("m2w", MLP, D)]
W1D = [("bq", D), ("bk", D), ("bv", D), ("bo", D), ("Eb", K), ("Fb", K),
       ("h1b", D), ("g1b", D), ("be1b", D), ("h2b", D), ("g2b", D), ("be2b", D),
       ("m1b", MLP), ("m2b", D)]

_cache = {}


def build():
    if "nc" in _cache:
        return _cache["nc"]
    nc = bacc.Bacc("TRN2", target_bir_lowering=False, debug=False, num_devices=8)
    ap = {}
    ap["x"] = nc.dram_tensor("x", [S, D], f32, kind="ExternalInput").ap()
    ap["z"] = nc.dram_tensor("z", [1, ZD], f32, kind="ExternalInput").ap()
    for nm, a, b in W2D:
        ap[nm] = nc.dram_tensor(nm, [a, b], f32, kind="ExternalInput").ap()
    for nm, a in W1D:
        ap[nm] = nc.dram_tensor(nm, [a], f32, kind="ExternalInput").ap()
    out = nc.dram_tensor("out", [S, D], f32, kind="ExternalOutput").ap()
    with tile.TileContext(nc, trace_sim=False) as tc:
        _emit(nc, tc, ap, out)
    nc.compile()
    _cache["nc"] = nc
    return nc


def _emit(nc, tc, ap, out):
    ctx = contextlib.ExitStack()
    with ctx:
        # ---------- whole-kernel pools ----------
        const = ctx.enter_context(tc.tile_pool(name="const", bufs=1))
        cols = ctx.enter_context(tc.tile_pool(name="cols", bufs=1))
        rows = ctx.enter_context(tc.tile_pool(name="rows", bufs=1))

        ident_f = const.tile([P, P], f32, tag="ident_f", name="ident_f")
        make_identity(nc, ident_f)
        ident_h = const.tile([P, P], f16, tag="ident_h", name="ident_h")
        nc.vector.tensor_copy(ident_h[:], ident_f[:])
        eps_t = const.tile([P, 1], f32, tag="eps", name="eps")
        nc.vector.memset(eps_t[:], EPS)
        ones1_h = const.tile([1, P], f16, tag="ones1_h", name="ones1_h")
        nc.vector.memset(ones1_h[:], 1.0)
        onescol_h = const.tile([P, 1], f16, tag="onescol_h", name="onescol_h")
        nc.vector.memset(onescol_h[:], 1.0)

        def col_load(name, n):
            t = cols.tile([P, n], f32, tag=f"cols_{name}", name=f"cols_{name}")
            for j in range(n):
                nc.sync.dma_start(t[:, j:j + 1], ap[name][ds(P * j, P)])
            return t

        def row_to_cols(tag, row_f, n=DC):
            cf = cols.tile([P, n], f32, tag=f"c_{tag}", name=f"c_{tag}")
            for j in range(n):
                nc.sync.dma_start(cf[:, j:j + 1], row_f[0:1, ds(P * j, P)])
            return cf

        def bcast_rows(tag, row_f, n, psp, pool, rpool):
            row_h = rpool.tile([1, n], f16, tag=f"rr_{tag}", name=f"rr_{tag}",
                               bufs=1)
            nc.vector.tensor_copy(row_h[:], row_f[0:1, 0:n])
            t = pool.tile([P, n], f32, tag=f"bc_{tag}", name=f"bc_{tag}")
            for h in range(0, n, GS):
                w = min(GS, n - h)
                pt = psp.tile([P, GS], f32, tag="bc_ps", name="bc_ps")
                nc.tensor.matmul(pt[:, 0:w], ones1_h[:], row_h[0:1, h:h + w],
                                 start=True, stop=True)
                nc.scalar.copy(t[:, h:h + w], pt[:, 0:w])
            return t

        # small, cheap DMAs first
        zc_f = cols.tile([P, DC], f32, tag="zc_f", name="zc_f")
        for j in range(DC):
            nc.sync.dma_start(zc_f[:, j:j + 1], ap["z"][0:1, ds(P * j, P)])
        zc_h = cols.tile([P, DC], f16, tag="zc_h", name="zc_h")
        nc.vector.tensor_copy(zc_h[:], zc_f[:])
        mv_t = cols.tile([P, SC, 2], f32, tag="mv", name="mv")

        # cross-phase stacks.  Right side LIFO (bottom->top):
        # aoT, qT, kv, pef, wkv.  Left: s_b{arow,x1T,wq,x1n} then
        # bc4, at, resw, C2-staging, MLP pools.
        s_resw = contextlib.ExitStack()
        s_b = contextlib.ExitStack()
        s_x1n = contextlib.ExitStack()
        s_wkv = contextlib.ExitStack()
        s_c2r = contextlib.ExitStack()    # right: aoT + qT + kv
        s_pef = contextlib.ExitStack()

        aoT_p = s_c2r.enter_context(tc.tile_pool(name="aoT", bufs=1,
                                                 side="right"))
        qT_p = s_c2r.enter_context(tc.tile_pool(name="qT", bufs=1,
                                                side="right"))
        kv_sb = s_c2r.enter_context(tc.tile_pool(name="kv_sb", bufs=1,
                                                 side="right"))
        pef_sb = s_pef.enter_context(
            tc.tile_pool(name="pef_sb", bufs=1, side="right"))
        arow = s_b.enter_context(tc.tile_pool(name="arow", bufs=1))
        x1T_p = s_b.enter_context(tc.tile_pool(name="x1T", bufs=1))
        wq_p = s_b.enter_context(tc.tile_pool(name="wq", bufs=1))
        vec_ctx = contextlib.ExitStack()
        vsb = vec_ctx.enter_context(tc.tile_pool(name="vec_sb", bufs=4))
        vps = vec_ctx.enter_context(
            tc.tile_pool(name="vec_ps", bufs=2, space="PSUM"))
        x1n_p = s_x1n.enter_context(tc.tile_pool(name="x1nat", bufs=1))

        qT = [[qT_p.tile([P, GS], f16, tag=f"qT_{j}_{g}", name=f"qT_{j}_{g}")
               for g in range(NG)] for j in range(DC)]
        pefEF = [pef_sb.tile([P, 2 * K], f16, tag=f"pef{j}", name=f"pef{j}")
                 for j in range(DC)]
        x1n = []

        def a_row_load(name, n):
            t = arow.tile([1, n], f32, tag="mrow", name=f"row_{name}", bufs=2)
            nc.sync.dma_start(t[:], ap[name][0:n])
            return t

        def a_half_load(name, h):
            t = arow.tile([1, GS], f32, tag="biash", name=f"rh_{name}{h}",
                          bufs=2)
            nc.sync.dma_start(t[:], ap[name][ds(GS * h, GS)])
            return t

        h1_row = arow.tile([1, D], f32, tag="hrow", name="h1", bufs=1)
        h2_row = arow.tile([1, D], f32, tag="hrow", name="h2", bufs=1)
        sc1_row = arow.tile([1, D], f32, tag="srow", name="sc1", bufs=2)
        of1_row = arow.tile([1, D], f32, tag="srow", name="of1", bufs=2)
        sc2_row = arow.tile([1, D], f32, tag="srow", name="sc2", bufs=2)
        of2_row = arow.tile([1, D], f32, tag="srow", name="of2", bufs=2)

        # ===== interleaved: LN1 groups + conditioning GEMV layers =====
        ln_ctx = contextlib.ExitStack()
        ln_sb = ln_ctx.enter_context(tc.tile_pool(name="ln1_sb", bufs=2))

        def ln1_group(g):
            for ii in range(4):
                i = 4 * g + ii
                xt = ln_sb.tile([P, D], f32, tag="ln_in", name="ln_in",
                                bufs=3)
                nc.sync.dma_start(xt[:], ap["x"][ds(P * i, P), :])
                st = ln_sb.tile([P, 2, 6], f32, tag="ln_st", name="ln_st")
                nc.vector.bn_stats(st[:, 0, :], xt[:, 0:GS])
                nc.vector.bn_stats(st[:, 1, :], xt[:, GS:D])
                mv = ln_sb.tile([P, 2], f32, tag="ln_mv", name="ln_mv")
                nc.vector.bn_aggr(mv[:], st[:])
                sd = ln_sb.tile([P, 1], f32, tag="ln_sd", name="ln_sd")
                nc.scalar.activation(sd[:], mv[:, 1:2], AF.Sqrt,
                                     bias=eps_t[:])
                rstd = ln_sb.tile([P, 1], f32, tag="ln_rstd", name="ln_rstd")
                nc.vector.reciprocal_approx_fast(rstd[:], sd[:])
                nmr = ln_sb.tile([P, 1], f32, tag="ln_nmr", name="ln_nmr")
                nc.vector.tensor_scalar(nmr[:], mv[:, 0:1], rstd[:],
                                        -1.0, OP.mult, OP.mult)
                x1t = x1n_p.tile([P, D], f16, tag=f"nat{i}", name=f"nat{i}")
                nc.scalar.activation(x1t[:], xt[:], AF.Identity,
                                     bias=nmr[:], scale=rstd[:])
                x1n.append(x1t)

        def vec_layer(wname, lhs_cols, bias_name, act, out_row, cast16):
            pts = [vps.tile([1, GS], f32, tag=f"vps{h}", name=f"vps{h}",
                            bufs=1) for h in range(2)]
            for j in range(DC):
                for h in range(2):
                    wt = vsb.tile([P, GS], f32, tag="vw_f", name="vw_f")
                    nc.sync.dma_start(wt[:],
                                      ap[wname][ds(P * j, P), ds(GS * h, GS)])
                    wh = vsb.tile([P, GS], f16, tag="vw_h", name="vw_h")
                    if (2 * j + h) % 2 == 0:
                        nc.vector.tensor_copy(wh[:], wt[:])
                    else:
                        nc.scalar.copy(wh[:], wt[:])
                    nc.tensor.matmul(pts[h][:], lhs_cols[:, j:j + 1],
                                     wh[:],
                                     start=(j == 0), stop=(j == DC - 1))
            for h in range(2):
                bias_h = a_half_load(bias_name, h)
                pre = arow.tile([1, GS], f32, tag=f"vpre{h}",
                                name=f"vpre{h}", bufs=1)
                nc.vector.tensor_add(pre[:], bias_h[:], pts[h][:])
                if act is None:
                    nc.vector.tensor_copy(out_row[0:1, ds(GS * h, GS)], pre[:])
                else:
                    nc.scalar.activation(out_row[0:1, ds(GS * h, GS)],
                                         pre[:], act)

        # g0 LN, then h1 chain, interleaving LN groups between layers
        ln1_group(0)
        vec_layer("h1w", zc_h, "h1b", AF.Silu, h1_row, False)
        h1_c = row_to_cols("h1", h1_row)
        h1_ch = cols.tile([P, DC], f16, tag="h1_ch", name="h1_ch")
        nc.vector.tensor_copy(h1_ch[:], h1_c[:])
        vec_layer("g1w", h1_ch, "g1b", None, sc1_row, True)
        vec_layer("be1w", h1_ch, "be1b", None, of1_row, True)
        ln1_group(1)
        rec1_row = arow.tile([1, D], f32, tag="mrow", name="rec1", bufs=2)
        nc.vector.reciprocal_approx_fast(rec1_row[:], sc1_row[:])
        o1s_row = arow.tile([1, D], f32, tag="mrow", name="o1s", bufs=2)
        nc.vector.tensor_mul(o1s_row[:], of1_row[:], rec1_row[:])
        s1_c = row_to_cols("s1", sc1_row)
        o1s_c = row_to_cols("o1s", o1s_row)
        o1s_ch = cols.tile([P, DC], f16, tag="o1s_ch", name="o1s_ch")
        nc.vector.tensor_copy(o1s_ch[:], o1s_c[:])
        ln1_group(2)
        ln1_group(3)
        ln_ctx.close()
        # wq resident with fused scale1 row-scaling (via vec staging)
        wq_r = []
        for j in range(DC):
            wr = wq_p.tile([P, D], f16, tag=f"wq_r{j}", name=f"wq_r{j}")
            for h in range(2):
                wt = vsb.tile([P, GS], f32, tag="vw_f", name="vw_f")
                nc.sync.dma_start(wt[:],
                                  ap["wq"][ds(P * j, P), ds(GS * h, GS)])
                nc.vector.tensor_scalar(wr[:, ds(GS * h, GS)], wt[:],
                                        s1_c[:, j:j + 1], None, OP.mult)
            wq_r.append(wr)

        # ---- PEF (two j-half passes, Ew|Fw re-streamed) + colsums ----
        cs_row = rows.tile([1, 2 * K], f32, tag="cs", name="cs")
        cs2_b = rows.tile([P, 2 * K], f32, tag="cs2b", name="cs2b")
        with (
            tc.tile_pool(name="ef_st", bufs=4) as efst,
            tc.tile_pool(name="pef_ps", bufs=1, space="PSUM") as pfps,
            tc.tile_pool(name="cs_ps", bufs=1, space="PSUM") as csps,
        ):
            cs_ps = csps.tile([1, 2 * K], f32, tag="cs_ps", name="cs_ps")
            for jh in range(2):
                pef_ps = [pfps.tile([P, 2 * K], f32, tag=f"pefp{jj}",
                                    name=f"pefp{jj}") for jj in range(4)]
                for i in range(SC):
                    ff = efst.tile([P, 2 * K], f32, tag="ef_f", name="ef_f")
                    nc.sync.dma_start(ff[:, 0:K], ap["Ew"][ds(P * i, P), :])
                    nc.sync.dma_start(ff[:, K:2 * K],
                                      ap["Fw"][ds(P * i, P), :])
                    fr = efst.tile([P, 2 * K], f16, tag="ef_r", name="ef_r",
                                   bufs=3)
                    nc.scalar.copy(fr[:], ff[:])
                    if jh == 0:
                        nc.tensor.matmul(cs_ps[:], onescol_h[:], fr[:],
                                         start=(i == 0), stop=(i == SC - 1))
                    for jj in range(4):
                        j = 4 * jh + jj
                        nc.tensor.matmul(pef_ps[jj][:],
                                         x1n[i][:, ds(P * j, P)], fr[:],
                                         start=(i == 0), stop=(i == SC - 1))
                if jh == 0:
                    nc.vector.tensor_copy(cs_row[:], cs_ps[:])
                    cs_row_h = arow.tile([1, 2 * K], f16, tag="cs_h",
                                         name="cs_h")
                    nc.vector.tensor_copy(cs_row_h[:], cs_row[:])
                    with tc.tile_pool(name="csb_ps", bufs=1,
                                      space="PSUM") as cbps:
                        cpt = cbps.tile([P, 2 * K], f32, tag="cs_bc",
                                        name="cs_bc")
                        nc.tensor.matmul(cpt[:], ones1_h[:], cs_row_h[:],
                                         start=True, stop=True)
                        nc.scalar.copy(cs2_b[:], cpt[:])
                for jj in range(4):
                    j = 4 * jh + jj
                    nc.vector.scalar_tensor_tensor(
                        pefEF[j][:], cs2_b[:], o1s_c[:, j:j + 1],
                        pef_ps[jj][:], OP.mult, OP.add)

        # ---- qT: transposes + bias GEMV + matmuls ----
        bq_eff_c = cols.tile([P, DC], f32, tag="bqe_c", name="bqe_c")
        with tc.tile_pool(name="q_ps", bufs=2, space="PSUM") as qps:
            bq_eff_row = arow.tile([1, D], f32, tag="mrow", name="bqe",
                                   bufs=2)
            gps = [qps.tile([1, GS], f32, tag=f"gv{h}", name=f"gv{h}",
                            bufs=1) for h in range(2)]
            for j in range(DC):
                for h in range(2):
                    nc.tensor.matmul(gps[h][:], o1s_ch[:, j:j + 1],
                                     wq_r[j][:, ds(GS * h, GS)],
                                     start=(j == 0), stop=(j == DC - 1))
            bq_row = a_row_load("bq", D)
            for h in range(2):
                nc.vector.tensor_add(bq_eff_row[0:1, ds(GS * h, GS)],
                                     bq_row[0:1, ds(GS * h, GS)], gps[h][:])
            for j in range(DC):
                nc.sync.dma_start(bq_eff_c[:, j:j + 1],
                                  bq_eff_row[0:1, ds(P * j, P)])
            with tc.tile_pool(name="tp1_ps2", bufs=2,
                              space="PSUM") as tpps:
                for g in range(NG):
                    x1T_g = [x1T_p.tile([P, GS], f16, tag=f"x1T_{j}",
                                        name=f"x1T_{j}", bufs=2)
                             for j in range(DC)]
                    for ii in range(4):
                        x1t = x1n[4 * g + ii]
                        for j in range(DC):
                            pt = tpps.tile([P, P], f16, tag="tp_ps",
                                           name="tp_ps")
                            nc.tensor.transpose(pt[:], x1t[:, ds(P * j, P)],
                                                ident_h[:])
                            if j % 2 == 0:
                                nc.vector.tensor_copy(
                                    x1T_g[j][:, ds(P * ii, P)], pt[:])
                            else:
                                nc.scalar.copy(
                                    x1T_g[j][:, ds(P * ii, P)], pt[:])
                    for jo in range(DC):
                        pt = qps.tile([P, GS], f32, tag="q_ps", name="q_ps")
                        for j in range(DC):
                            nc.tensor.matmul(pt[:], wq_r[j][:, ds(P * jo, P)],
                                             x1T_g[j][:],
                                             start=(j == 0),
                                             stop=(j == DC - 1))
                        nc.scalar.activation(qT[jo][g][:], pt[:], AF.Identity,
                                             bias=bq_eff_c[:, jo:jo + 1])
        s_x1n.close()
        # late conditioning chain (scale2/offset2) overlaps qT on DMA
        vec_layer("h2w", zc_h, "h2b", AF.Silu, h2_row, False)
        h2_c = row_to_cols("h2", h2_row)
        h2_ch = cols.tile([P, DC], f16, tag="h2_ch", name="h2_ch")
        nc.vector.tensor_copy(h2_ch[:], h2_c[:])
        vec_layer("g2w", h2_ch, "g2b", None, sc2_row, True)
        vec_layer("be2w", h2_ch, "be2b", None, of2_row, True)
        rec2_row = arow.tile([1, D], f32, tag="mrow", name="rec2", bufs=2)
        nc.vector.reciprocal_approx_fast(rec2_row[:], sc2_row[:])
        o2s_row = arow.tile([1, D], f32, tag="mrow", name="o2s", bufs=2)
        nc.vector.tensor_mul(o2s_row[:], of2_row[:], rec2_row[:])
        s2_c = row_to_cols("s2", sc2_row)
        o2s_c = row_to_cols("o2s", o2s_row)
        vec_ctx.close()
        s_b.close()   # frees arow, x1T, wq

        # wk/wv residents (right stack top, freed after KV)
        wkv_p = s_wkv.enter_context(tc.tile_pool(name="wkv", bufs=1,
                                                 side="right"))
        wk_r, wv_r = [], []
        with tc.tile_pool(name="wkv_st", bufs=3) as wkst:
            for nm, lst in (("wk", wk_r), ("wv", wv_r)):
                for j in range(DC):
                    wt = wkst.tile([P, D], f32, tag="wkv_f", name="wkv_f")
                    nc.sync.dma_start(wt[:], ap[nm][ds(P * j, P), :])
                    wr = wkv_p.tile([P, D], f16, tag=f"{nm}_r{j}",
                                    name=f"{nm}_r{j}")
                    nc.vector.tensor_scalar(wr[:], wt[:], s1_c[:, j:j + 1],
                                            None, OP.mult)
                    lst.append(wr)

        # ===== phase KV: kpT + vpe (with fused ones cols) =====
        kpT = [kv_sb.tile([P, K], f16, tag=f"kpT{j}", name=f"kpT{j}")
               for j in range(DC)]
        vpe = [kv_sb.tile([P, P * H], f16, tag=f"vpe{c}", name=f"vpe{c}")
               for c in range(KC)]
        with (
            tc.tile_pool(name="kv_bias", bufs=1) as kvb,
            tc.tile_pool(name="kv_tmp", bufs=2) as kvt,
            tc.tile_pool(name="kv_ps", bufs=2, space="PSUM") as kvps,
        ):
            def kv_row_load(name, n):
                t = kvb.tile([1, n], f32, tag=f"kvrow_{name}",
                             name=f"kvrow_{name}")
                nc.sync.dma_start(t[:], ap[name][0:n])
                return t

            bk_c = col_load("bk", DC)
            Fb_c = col_load("Fb", KC)
            bv_row = kv_row_load("bv", D)
            Eb_row = kv_row_load("Eb", K)
            with tc.tile_pool(name="kv_bc_ps", bufs=2, space="PSUM") as kbps:
                bv_b = bcast_rows("bv", bv_row, D, kbps, kvb, kvt)
                Eb_b = bcast_rows("Eb", Eb_row, K, kbps, kvb, kvt)
            csF_c = kvb.tile([P, KC], f32, tag="csF_c", name="csF_c")
            for c in range(KC):
                nc.sync.dma_start(csF_c[:, c:c + 1],
                                  cs_row[0:1, ds(K + P * c, P)])
            kp_bias = []
            for j in range(DC):
                bt = kvb.tile([P, K], f32, tag=f"kpb{j}", name=f"kpb{j}")
                nc.vector.tensor_scalar(bt[:], cs2_b[:, 0:K], bk_c[:, j:j + 1],
                                        None, OP.mult)
                nc.vector.tensor_add(bt[:], bt[:], Eb_b[:])
                kp_bias.append(bt)
            vp_bias = []
            for c in range(KC):
                bt = kvb.tile([P, D], f32, tag=f"vpb{c}", name=f"vpb{c}")
                nc.vector.tensor_scalar(bt[:], bv_b[:], csF_c[:, c:c + 1],
                                        Fb_c[:, c:c + 1], OP.mult, OP.add)
                vp_bias.append(bt)

            for c in range(KC):
                for h in range(H):
                    nc.vector.memset(vpe[c][:, ds(P * h + DH, DH)], 1.0)
            for jo in range(DC):
                pt = kvps.tile([P, K], f32, tag="kp_ps", name="kp_ps")
                for j in range(DC):
                    nc.tensor.matmul(pt[:], wk_r[j][:, ds(P * jo, P)],
                                     pefEF[j][:, 0:K],
                                     start=(j == 0), stop=(j == DC - 1))
                nc.vector.tensor_add(kpT[jo][:], kp_bias[jo][:], pt[:])
            for hf in range(2):
                for c in range(KC):
                    pt = kvps.tile([P, GS], f32, tag="vp_ps", name="vp_ps")
                    for j in range(DC):
                        nc.tensor.matmul(pt[:],
                                         pefEF[j][:, ds(K + P * c, P)],
                                         wv_r[j][:, ds(GS * hf, GS)],
                                         start=(j == 0), stop=(j == DC - 1))
                    tmp = kvt.tile([P, GS], f32, tag="vp_tmp", name="vp_tmp")
                    nc.vector.tensor_add(tmp[:],
                                         vp_bias[c][:, ds(GS * hf, GS)], pt[:])
                    for hh in range(8):
                        h = 8 * hf + hh
                        if hh % 2 == 0:
                            nc.vector.tensor_copy(vpe[c][:, ds(P * h, DH)],
                                                  tmp[:, ds(DH * hh, DH)])
                        else:
                            nc.scalar.copy(vpe[c][:, ds(P * h, DH)],
                                           tmp[:, ds(DH * hh, DH)])
        s_wkv.close()
        s_pef.close()

        # left-side post-B long-lived pools + wo resident
        bc4 = ctx.enter_context(tc.tile_pool(name="bc4", bufs=1))
        at_p = ctx.enter_context(tc.tile_pool(name="at", bufs=1))
        resw = s_resw.enter_context(tc.tile_pool(name="resw", bufs=1))
        wo_r = []
        with tc.tile_pool(name="wo_st", bufs=3) as wost:
            with tc.tile_pool(name="wo_bc_ps", bufs=2, space="PSUM") as wbps:
                bo_row = wost.tile([1, D], f32, tag="bo_row", name="bo_row",
                                   bufs=1)
                nc.sync.dma_start(bo_row[:], ap["bo"][0:D])
                m2b_row = wost.tile([1, D], f32, tag="m2b_row",
                                    name="m2b_row", bufs=1)
                nc.sync.dma_start(m2b_row[:], ap["m2b"][0:D])
                bo_b = bcast_rows("bo", bo_row, D, wbps, bc4, wost)
                m2b_b = bcast_rows("m2b", m2b_row, D, wbps, bc4, wost)
            for j in range(DC):
                wt = wost.tile([P, D], f32, tag="wo_f", name="wo_f")
                nc.sync.dma_start(wt[:], ap["wo"][ds(P * j, P), :])
                wr = resw.tile([P, D], f16, tag=f"wo_r{j}", name=f"wo_r{j}")
                if j % 2 == 0:
                    nc.scalar.copy(wr[:], wt[:])
                else:
                    nc.vector.tensor_copy(wr[:], wt[:])
                wo_r.append(wr)

        # ===== C2 + C3 fused per token group =====
        aoT = [[aoT_p.tile([P, GS], f16, tag=f"aoT_{j}_{g}",
                           name=f"aoT_{j}_{g}")
                for g in range(NG)] for j in range(DC)]
        at = [at_p.tile([P, D], f32, tag=f"at{i}", name=f"at{i}")
              for i in range(SC)]
        with (
            tc.tile_pool(name="at_sb", bufs=8) as atsb,
            tc.tile_pool(name="den_sb", bufs=2) as densb,
            tc.tile_pool(name="c3_sb", bufs=3) as c3sb,
            tc.tile_pool(name="sc_ps", bufs=3, space="PSUM") as scps,
            tc.tile_pool(name="av_ps", bufs=3, space="PSUM") as avps,
            tc.tile_pool(name="wo_ps", bufs=2, space="PSUM") as wops,
        ):
            def emit_scores_pair(g, p):
                exps = [[None, None], [None, None]]
                for c in range(KC):
                    for e in range(2):
                        r0 = 64 * e
                        spt = scps.tile([P, GS], f32, tag="sc", name="sc")
                        nc.tensor.matmul(spt[:],
                                         kpT[p][r0:r0 + 64, ds(P * c, P)],
                                         qT[p][g][r0:r0 + 64, :],
                                         start=True, stop=True)
                        et = atsb.tile([P, GS], f16, tag="exp", name="exp")
                        nc.scalar.activation(et[:], spt[:], AF.Exp,
                                             scale=0.125)
                        exps[e][c] = et
                return exps

            for g in range(NG):
                exps = emit_scores_pair(g, 0)
                for p in range(8):
                    nxt = emit_scores_pair(g, p + 1) if p < 7 else None
                    apts = []
                    for e in range(2):
                        h = 2 * p + e
                        apt = avps.tile([P, GS], f32, tag="av", name="av")
                        for c in range(KC):
                            nc.tensor.matmul(apt[:], vpe[c][:, ds(P * h, P)],
                                             exps[e][c][:],
                                             start=(c == 0),
                                             stop=(c == KC - 1))
                        apts.append(apt)
                    den = densb.tile([64, 2 * GS], f32, tag="den", name="den")
                    nc.scalar.copy(den[:, 0:GS], apts[0][64:128, :])
                    nc.vector.tensor_copy(den[:, GS:2 * GS],
                                          apts[1][64:128, :])
                    rec = densb.tile([64, 2 * GS], f32, tag="rec", name="rec",
                                     bufs=1)
                    nc.vector.reciprocal_approx_fast(rec[:], den[:])
                    nc.vector.tensor_mul(aoT[p][g][0:64, :],
                                         apts[0][0:64, :], rec[:, 0:GS])
                    nc.vector.tensor_mul(aoT[p][g][64:128, :],
                                         apts[1][0:64, :], rec[:, GS:2 * GS])
                    exps = nxt
                for ii in range(4):
                    i = 4 * g + ii
                    xt2 = c3sb.tile([P, D], f32, tag="res_x", name="res_x",
                                    bufs=3)
                    nc.sync.dma_start(xt2[:], ap["x"][ds(P * i, P), :])
                    for half in range(2):
                        pt = wops.tile([P, GS], f32, tag="wo_ps", name="wo_ps")
                        for j in range(DC):
                            nc.tensor.matmul(pt[:],
                                             aoT[j][g][:, ds(P * ii, P)],
                                             wo_r[j][:, ds(GS * half, GS)],
                                             start=(j == 0), stop=(j == DC - 1))
                        tm = c3sb.tile([P, GS], f32, tag="wo_tmp",
                                       name="wo_tmp", bufs=2)
                        nc.vector.tensor_add(tm[:], bo_b[:, ds(GS * half, GS)],
                                             pt[:])
                        nc.gpsimd.tensor_add(at[i][:, ds(GS * half, GS)],
                                             tm[:], xt2[:, ds(GS * half, GS)])
                    st = c3sb.tile([P, 2, 6], f32, tag="c3_st", name="c3_st")
                    nc.vector.bn_stats(st[:, 0, :], at[i][:, 0:GS])
                    nc.vector.bn_stats(st[:, 1, :], at[i][:, GS:D])
                    nc.vector.bn_aggr(mv_t[:, i, :], st[:])
        s_resw.close()
        s_c2r.close()

        # ===== LN2 + MLP, two supergroups of 1024 tokens =====
        m1b_c = col_load("m1b", MC)
        for sg in range(2):
            sctx = contextlib.ExitStack()
            with sctx:
                x2T_p = sctx.enter_context(tc.tile_pool(name="x2T", bufs=1))
                hm_p = sctx.enter_context(tc.tile_pool(name="hm", bufs=1))
                x2T = [x2T_p.tile([P, 2 * GS], f16, tag=f"x2T_{j}",
                                  name=f"x2T_{j}") for j in range(DC)]
                hm = [hm_p.tile([P, 2 * GS], f16, tag=f"hm{m}", name=f"hm{m}")
                      for m in range(MC)]
                with (
                    tc.tile_pool(name="ln2_sb", bufs=2) as ln2sb,
                    tc.tile_pool(name="tp2_ps", bufs=2, space="PSUM") as tp2ps,
                ):
                    for gg in range(2):
                        g = 2 * sg + gg
                        sd4 = ln2sb.tile([P, 4], f32, tag="l2sd", name="l2sd")
                        for ii in range(4):
                            i = 4 * g + ii
                            nc.scalar.activation(sd4[:, ii:ii + 1],
                                                 mv_t[:, i, 1:2],
                                                 AF.Sqrt, bias=eps_t[:])
                        rstd4 = ln2sb.tile([P, 4], f32, tag="l2rs",
                                           name="l2rs")
                        nc.vector.reciprocal_approx_fast(rstd4[:], sd4[:])
                        for ii in range(4):
                            i = 4 * g + ii
                            rstd = rstd4[:, ii:ii + 1]
                            nmr = ln2sb.tile([P, 1], f32, tag="l2nm",
                                             name="l2nm")
                            nc.vector.tensor_scalar(nmr[:], mv_t[:, i, 0:1],
                                                    rstd, -1.0,
                                                    OP.mult, OP.mult)
                            x2t = ln2sb.tile([P, D], f16, tag="x2nat",
                                             name="x2nat")
                            nc.scalar.activation(x2t[:], at[i][:],
                                                 AF.Identity,
                                                 bias=nmr[:], scale=rstd)
                            tl = GS * gg + P * ii
                            for j in range(DC):
                                pt = tp2ps.tile([P, P], f16, tag="tp2",
                                                name="tp2")
                                nc.tensor.transpose(pt[:],
                                                    x2t[:, ds(P * j, P)],
                                                    ident_h[:])
                                nc.scalar.activation(x2T[j][:, ds(tl, P)],
                                                     pt[:], AF.Identity,
                                                     bias=o2s_c[:, j:j + 1])
                with (
                    tc.tile_pool(name="m1_st", bufs=8) as m1st,
                    tc.tile_pool(name="m1h_st", bufs=8) as m1hst,
                    tc.tile_pool(name="m1_ps", bufs=2, space="PSUM") as m1ps,
                ):
                    for mq in range(8):
                        w1h = []
                        for j in range(DC):
                            wf = m1st.tile([P, GS], f32, tag="m1f",
                                           name="m1f")
                            nc.sync.dma_start(
                                wf[:],
                                ap["m1w"][ds(P * j, P), ds(GS * mq, GS)])
                            wh = m1hst.tile([P, GS], f16, tag="m1h",
                                            name="m1h")
                            if j % 2 == 0:
                                nc.vector.tensor_scalar(
                                    wh[:], wf[:], s2_c[:, j:j + 1], None,
                                    OP.mult)
                            else:
                                nc.scalar.activation(
                                    wh[:], wf[:], AF.Copy,
                                    scale=s2_c[:, j:j + 1])
                            w1h.append(wh)
                        for mm in range(4):
                            m = 4 * mq + mm
                            for g2 in range(2):
                                pt = m1ps.tile([P, GS], f32, tag="m1p",
                                               name="m1p")
                                for j in range(DC):
                                    nc.tensor.matmul(
                                        pt[:], w1h[j][:, ds(P * mm, P)],
                                        x2T[j][:, ds(GS * g2, GS)],
                                        start=(j == 0), stop=(j == DC - 1))
                                nc.scalar.activation(
                                    hm[m][:, ds(GS * g2, GS)], pt[:],
                                    AF.Gelu, bias=m1b_c[:, m:m + 1])
                with (
                    tc.tile_pool(name="m2_st", bufs=6) as m2st,
                    tc.tile_pool(name="m2h_st", bufs=5) as m2hst,
                    tc.tile_pool(name="e_sb", bufs=3) as esb,
                    tc.tile_pool(name="m2_ps", bufs=1, space="PSUM") as m2ps,
                ):
                    for half in range(2):
                        m2p = [m2ps.tile([P, GS], f32, tag=f"m2p{ss}",
                                         name=f"m2p{ss}") for ss in range(8)]
                        for m in range(MC):
                            wf = m2st.tile([P, GS], f32, tag="m2f",
                                           name="m2f")
                            nc.sync.dma_start(
                                wf[:],
                                ap["m2w"][ds(P * m, P), ds(GS * half, GS)])
                            wh = m2hst.tile([P, GS], f16, tag="m2h",
                                            name="m2h")
                            if m % 2 == 0:
                                nc.vector.tensor_copy(wh[:], wf[:])
                            else:
                                nc.scalar.copy(wh[:], wf[:])
                            for ss in range(8):
                                nc.tensor.matmul(
                                    m2p[ss][:], hm[m][:, ds(P * ss, P)],
                                    wh[:],
                                    start=(m == 0), stop=(m == MC - 1))
                        for ss in range(8):
                            i = 8 * sg + ss
                            tm = esb.tile([P, GS], f32, tag="e_tmp",
                                          name="e_tmp")
                            nc.vector.tensor_add(
                                tm[:], m2b_b[:, ds(GS * half, GS)],
                                m2p[ss][:])
                            ot = esb.tile([P, GS], f32, tag="e_out",
                                          name="e_out")
                            nc.gpsimd.tensor_add(
                                ot[:], tm[:], at[i][:, ds(GS * half, GS)])
                            nc.sync.dma_start(
                                out[ds(P * i, P), ds(GS * half, GS)], ot[:])


def kernel(**inputs):
    nc = build()
    x = np.ascontiguousarray(inputs["x"], dtype=np.float32)
    z = np.ascontiguousarray(inputs["z"], dtype=np.float32)
    base = {}
    for nm, _, _ in W2D:
        base[nm] = np.ascontiguousarray(inputs[nm], dtype=np.float32)
    for nm, _ in W1D:
        base[nm] = np.ascontiguousarray(inputs[nm], dtype=np.float32)
    in_maps = []
    for c in range(B):
        m = dict(base)
        m["x"] = x[c]
        m["z"] = z[c:c + 1]
        in_maps.append(m)
    res = run_bass_kernel_spmd(nc, in_maps, list(range(B)))
    _cache["last"] = res
    return np.stack([res.results[c]["out"] for c in range(B)], axis=0)
